# revision 1
# baseline (speedup 1.0000x reference)
"""Trainium2 Bass kernel for the FFTBlock problem (B=2, C=32, H=2688, W=128).

Math (reference):
  spatial  = relu(conv7x1_s7(x) + b_spatial)                        [B,C,384,W]
  spectral = irfft(relu(w_spectral @ rfft_concat(x) + b_spectral))  per (b,c,w)
  out = spatial + spectral

Transformation: rfft/irfft along H are linear maps, so with
  F: rfft matrix (imag-DC and imag-Nyquist rows are zero and dropped -> 2688
     rows), A = w_spectral @ F : [384, 2688] (output channels 193/385 =
     imag-DC/imag-Nyquist are multiplied by zero irfft columns and dropped ->
     384), G: [384, 384] irfft matrix (same columns dropped):
  spectral_col = G @ relu(A @ x_col + b)

Device plan (8 NeuronCores, W sharded 8 x 16):
  Launch 1 "fold":  core i computes A^T[336*i:336*(i+1), :] =
                    F[:, hslice]^T @ w_spectral^T; host concatenates.
  Launch 2 "main":  per core: GEMM1 conv[384, 1024] = A @ x_cols (fp32r),
                    bias+relu (ACT), GEMM2 spec[384, 1024] = G @ relu (fp32r),
                    spatial conv as GEMM [32, 224] @ [224, 12288] (bf16 - the
                    spatial branch is small-magnitude; bf16 error is negligible
                    vs output absmax), reorg via DRAM bounce, on-chip add.
fp32r runs at full PE rate for free dim >= 256, with ~1e-4 relative error.
"""

import os

import numpy as np
import ml_dtypes

import contextlib

import concourse.bacc as bacc
import concourse.mybir as mybir
import concourse.tile as tile
from concourse.bass_utils import run_bass_kernel_spmd
from concourse.alu_op_type import AluOpType


def _maybe_loop(tc, n):
    return tc.For_i(0, n, 1) if n > 1 else contextlib.nullcontext()

N_CORES = 8
B, C, H, W = 2, 32, 2688, 128
FREQ_IN = H // 2 + 1            # 1345
KF = H                          # 2688 usable rfft rows (2 zero rows dropped)
OUT_H = 384
FREQ_OUT = OUT_H // 2 + 1       # 193
MO = 2 * FREQ_OUT - 2           # 384 usable conv channels (2 dead dropped)
WS = W // N_CORES               # 16 width columns per core
NCOL = B * C * WS               # 1024 spectral columns per core
NSP = B * OUT_H * WS            # 12288 spatial columns per core
KSP = C * 7                     # 224 spatial reduction
HSL = H // N_CORES              # 336 fold output rows per core

F32 = mybir.dt.float32
F32R = mybir.dt.float32r
BF16 = mybir.dt.bfloat16
F16 = mybir.dt.float16
RELU = mybir.ActivationFunctionType.Relu
COPY = mybir.ActivationFunctionType.Copy

_cache = {}
LAST_EXEC_NS = None
LAST_FOLD_NS = None


def _trace_flag():
    return bool(int(os.environ.get("KERNEL_TRACE", "0")))


def _dft_constants():
    """F [2688, 2688] (rfft, ortho, dead rows dropped) and G [384, 384]
    (irfft, ortho, dead cols dropped)."""
    if "F" in _cache:
        return _cache["F"], _cache["G"]
    Fc = np.fft.rfft(np.eye(H), axis=0, norm="ortho")       # [1345, 2688]
    F = np.concatenate([Fc.real, Fc.imag[1:FREQ_IN - 1]], axis=0)
    F = np.ascontiguousarray(F, dtype=np.float32)           # [2688, 2688]
    G_re = np.fft.irfft(np.eye(FREQ_OUT), n=OUT_H, axis=0, norm="ortho")
    G_im = np.fft.irfft(1j * np.eye(FREQ_OUT), n=OUT_H, axis=0, norm="ortho")
    G = np.concatenate([G_re, G_im[:, 1:FREQ_OUT - 1]], axis=1)
    G = np.ascontiguousarray(G, dtype=np.float32)           # [384, 384]
    _cache["F"] = F
    _cache["G"] = G
    return F, G


def _spec_keep_idx():
    """Kept rfft rows (of the 2690 concat) / output channels (of the 386)."""
    keep_f = list(range(FREQ_IN)) + [FREQ_IN + k for k in range(1, FREQ_IN - 1)]
    keep_o = list(range(FREQ_OUT)) + [FREQ_OUT + k for k in range(1, FREQ_OUT - 1)]
    return np.array(keep_f), np.array(keep_o)


def _build_fold(loop_n=1):
    """Per core: at_sl[336, 384] = f_sl[2688, 336]^T @ w_t[2688, 384]."""
    key = ("fold", loop_n)
    if key in _cache:
        return _cache[key]
    nc = bacc.Bacc("TRN2", target_bir_lowering=False, debug=False,
                   num_devices=N_CORES)
    f_sl = nc.dram_tensor("f_sl", [KF, HSL], F16, kind="ExternalInput").ap()
    w_t = nc.dram_tensor("w_t", [KF, MO], F16, kind="ExternalInput").ap()
    at_sl = nc.dram_tensor("at_sl", [HSL, MO], F32, kind="ExternalOutput").ap()

    KT = KF // 128               # 21
    MT = (HSL + 127) // 128      # 3 (128, 128, 80)

    with tile.TileContext(nc) as tc:
        with tc.tile_pool(name="w", bufs=1) as wp, \
             tc.tile_pool(name="f", bufs=1) as fp, \
             tc.tile_pool(name="o", bufs=2) as op, \
             tc.tile_pool(name="ps", bufs=1, space="PSUM") as pp:
            # batched loads: one DMA covers several 128-row k-tiles, laid
            # side by side in the free dim of one wide SBUF tile
            CH = 7                      # k-tiles per DMA
            wt, ft = [], []
            for g in range(KT // CH):
                wg = wp.tile([128, CH * MO], F16, tag=f"wg{g}", name=f"wg{g}")
                nc.sync.dma_start(
                    wg[:], w_t[128 * CH * g:128 * CH * (g + 1), :]
                    .rearrange("(k p) m -> p k m", p=128))
                fg = fp.tile([128, CH * HSL], F16, tag=f"fg{g}", name=f"fg{g}")
                nc.sync.dma_start(
                    fg[:], f_sl[128 * CH * g:128 * CH * (g + 1), :]
                    .rearrange("(k p) m -> p k m", p=128))
                for j in range(CH):
                    wt.append(wg[:, MO * j:MO * (j + 1)])
                    ft.append(fg[:, HSL * j:HSL * (j + 1)])
            for m in range(MT):
                mp = min(128, HSL - 128 * m)
                ps = pp.tile([mp, MO], F32, tag="ps", name="ps")
                for k in range(KT):
                    nc.tensor.matmul(ps[:], ft[k][:, 128 * m:128 * m + mp],
                                     wt[k], start=(k == 0), stop=(k == KT - 1))
                ot = op.tile([mp, MO], F32, tag="ot", name="ot")
                nc.scalar.activation(ot[:], ps[:], COPY)
                nc.sync.dma_start(at_sl[128 * m:128 * m + mp, :], ot[:])
    nc.compile()
    _cache[key] = nc
    return nc


def _build_main(loop_n=1):
    """Per core main kernel; out[384, 1024] in (t, (b, c, w)) layout."""
    key = ("main", loop_n)
    if key in _cache:
        return _cache[key]
    nc = bacc.Bacc("TRN2", target_bir_lowering=False, debug=False,
                   num_devices=N_CORES)
    at = nc.dram_tensor("at", [H, MO], F16, kind="ExternalInput").ap()
    xt = nc.dram_tensor("xt", [H, NCOL], F16, kind="ExternalInput").ap()
    gt = nc.dram_tensor("gt", [MO, OUT_H], F16, kind="ExternalInput").ap()
    bspec = nc.dram_tensor("bspec", [MO, 1], F32, kind="ExternalInput").ap()
    wsp = nc.dram_tensor("wsp", [KSP, C], F16, kind="ExternalInput").ap()
    bsp = nc.dram_tensor("bsp", [C, 1], F32, kind="ExternalInput").ap()
    xsp = nc.dram_tensor("xsp", [KSP, NSP], F16, kind="ExternalInput").ap()
    out = nc.dram_tensor("out", [OUT_H, NCOL], F32, kind="ExternalOutput").ap()

    KT1 = H // 128               # 21 k-tiles for GEMM1
    MT1 = MO // 128              # 3 m-tiles
    NT = NCOL // 512             # 2 n-tiles
    MT2 = OUT_H // 128           # 3 m-tiles for GEMM2
    NSPC = NSP // 512            # 24 spatial chunks
    TCH = 512 // WS              # 32 t values per spatial chunk

    with tile.TileContext(nc) as tc:
        with tc.tile_pool(name="const", bufs=1) as cst, \
             tc.tile_pool(name="xtp", bufs=1) as xtp, \
             tc.tile_pool(name="xspp", bufs=1) as xspp, \
             tc.tile_pool(name="relu", bufs=1) as rlp, \
             tc.tile_pool(name="spsb", bufs=3) as spsb, \
             tc.tile_pool(name="outp", bufs=1) as outp, \
             tc.tile_pool(name="ps_g1", bufs=1, space="PSUM") as psg1, \
             _maybe_loop(tc, loop_n):

            # ---- weights first (small, batched DMAs), then xt stream ----
            CH = 7                      # k-tiles per batched DMA
            at_t = []
            at_dmas = []
            for g in range(KT1 // CH):
                ag = cst.tile([128, CH * MO], F16, tag=f"atg{g}", name=f"atg{g}")
                at_dmas.append((ag, g))
                for j in range(CH):
                    at_t.append(ag[:, MO * j:MO * (j + 1)])
            gt_big = cst.tile([128, MT2 * OUT_H], F16, tag="gt_big", name="gt_big")
            nc.sync.dma_start(gt_big[:],
                              gt[:].rearrange("(k p) m -> p k m", p=128))
            gt_t = [gt_big[:, OUT_H * k:OUT_H * (k + 1)] for k in range(MT2)]
            bspec_big = cst.tile([128, MT1], F32, tag="bspec_big", name="bspec_big")
            nc.sync.dma_start(bspec_big[:],
                              bspec[:].rearrange("(m p) one -> p m one", p=128))
            bspec_t = [bspec_big[:, m:m + 1] for m in range(MT1)]
            wsp1 = cst.tile([128, C], F16, tag="wsp1", name="wsp1")
            nc.sync.dma_start(wsp1[:], wsp[0:128, :])
            wsp2 = cst.tile([KSP - 128, C], F16, tag="wsp2", name="wsp2")
            nc.sync.dma_start(wsp2[:], wsp[128:KSP, :])
            bsp_t = cst.tile([C, 1], F32, tag="bsp", name="bsp")
            nc.sync.dma_start(bsp_t[:], bsp[:])

            XCH = 3                     # xt k-tiles per DMA
            xt_t = []
            xt_tiles = []
            for g in range(KT1 // XCH):
                xg = xtp.tile([128, XCH * NCOL], F16, tag=f"xtg{g}", name=f"xtg{g}")
                xt_tiles.append(xg)
                for j in range(XCH):
                    xt_t.append(xg[:, NCOL * j:NCOL * (j + 1)])
            def emit_at_dma(g):
                ag = at_dmas[g][0]
                nc.sync.dma_start(
                    ag[:], at[128 * CH * g:128 * CH * (g + 1), :]
                    .rearrange("(k p) m -> p k m", p=128))

            def emit_xt_dma(g):
                nc.sync.dma_start(
                    xt_tiles[g][:], xt[128 * XCH * g:128 * XCH * (g + 1), :]
                    .rearrange("(k p) m -> p k m", p=128))

            for kind, g in [("at", 0), ("xt", 0), ("xt", 1), ("at", 1),
                            ("xt", 2), ("at", 2), ("xt", 3)]:
                (emit_at_dma if kind == "at" else emit_xt_dma)(g)

            # ---- GEMM1: conv[384, 1024] = A @ x; m-outer, k-inner, both
            # n-slices per weight load ----
            relu_t = []
            for m in range(MT1):
                rt = rlp.tile([128, NCOL], F16, tag=f"relu{m}", name=f"relu{m}")
                relu_t.append(rt)
            # spatial branch: scatter straight into `out` (spec layout);
            # the spectral result is accumulated on top via CCE accum DMA
            out_tcw = out.rearrange("t (b c w) -> t b c w", b=B, c=C)
            GRP = 6                       # 512-col chunks per xsp load group
            NGRP = NSPC // GRP            # 4 groups
            GW = GRP * 512                # 3072 cols per group

            xsp_tiles = {}

            def xsp_load(gi):
                x1 = xspp.tile([128, GW], F16, tag=f"xsp1_{gi}", name=f"xsp1_{gi}")
                nc.sync.dma_start(x1[:], xsp[0:128, GW * gi:GW * (gi + 1)])
                x2 = xspp.tile([KSP - 128, GW], F16, tag=f"xsp2_{gi}",
                               name=f"xsp2_{gi}")
                nc.sync.dma_start(x2[:], xsp[128:KSP, GW * gi:GW * (gi + 1)])
                xsp_tiles[gi] = (x1, x2)

            def spatial_group(gi):
                x1, x2 = xsp_tiles[gi]
                sp = spsb.tile([C, GW], F32, tag="sp", name="sp")
                for j in range(GRP):
                    jsl = slice(512 * j, 512 * (j + 1))
                    ps = psg1.tile([C, 512], F32,
                                   tag=f"g1m{j % MT1}n{j % NT}", name="ps_sp")
                    nc.tensor.matmul(ps[:], wsp1[:], x1[:, jsl], start=True, stop=False)
                    nc.tensor.matmul(ps[:], wsp2[:], x2[:, jsl], start=False, stop=True)
                    if j % 2 == 0:
                        nc.scalar.activation(sp[:, jsl], ps[:], RELU, bias=bsp_t[:])
                    else:
                        # relu(x + b) fused on DVE: (ps + bias) max 0
                        nc.vector.tensor_scalar(sp[:, jsl], ps[:], bsp_t[:], 0.0,
                                                AluOpType.add, AluOpType.max)
                # group gi covers b = gi // 2, t-range of 192
                b_i = gi // (NGRP // B)
                t0 = (GRP * TCH) * (gi % (NGRP // B))
                dst = out_tcw[t0:t0 + GRP * TCH, b_i, :, :].transpose([1, 0, 2])
                nc.sync.dma_start(dst, sp[:].rearrange("c (t w) -> c t w", w=WS))

            # queue remaining xt loads, then all xsp loads
            for g in (4, 5, 6):
                emit_xt_dma(g)
            for gi in range(NGRP):
                xsp_load(gi)

            # GEMM1 k-outer over all (m, n): each xt k-tile is fully
            # consumed on arrival (6 matmuls), PE stays dense and warm
            ps_mn = {}
            for m in range(MT1):
                for n in range(NT):
                    ps_mn[(m, n)] = psg1.tile([128, 512], F32,
                                              tag=f"g1m{m}n{n}", name=f"g1m{m}n{n}")
            for k in range(KT1):
                for m in range(MT1):
                    msl = slice(128 * m, 128 * (m + 1))
                    for n in range(NT):
                        nc.tensor.matmul(ps_mn[(m, n)][:], at_t[k][:, msl],
                                         xt_t[k][:, 512 * n:512 * (n + 1)],
                                         start=(k == 0), stop=(k == KT1 - 1))
            for m in range(MT1):
                for n in range(NT):
                    nc.scalar.activation(relu_t[m][:, 512 * n:512 * (n + 1)],
                                         ps_mn[(m, n)][:], RELU, bias=bspec_t[m][:])
            g2_pairs = [(m2, n) for m2 in range(MT2) for n in range(NT)]

            so_t = {}

            def gemm2_pair(m2, n):
                m2sl = slice(128 * m2, 128 * (m2 + 1))
                t0 = 128 * m2
                nsl = slice(512 * n, 512 * (n + 1))
                ps2 = psg1.tile([128, 512], F32, tag=f"g1m{m2}n{n}", name="g2")
                for k in range(MT2):
                    nc.tensor.matmul(ps2[:], gt_t[k][:, m2sl],
                                     relu_t[k][:, nsl],
                                     start=(k == 0), stop=(k == MT2 - 1))
                if m2 not in so_t:
                    so_t[m2] = outp.tile([128, NCOL], F32, tag=f"so{m2}",
                                         name=f"so{m2}")
                nc.vector.tensor_copy(so_t[m2][:, nsl], ps2[:])
                if n == NT - 1:
                    nc.gpsimd.dma_start(out[t0:t0 + 128, :], so_t[m2][:],
                                        accum_op=AluOpType.add)

            # an accumulate may only run after the spatial scatters covering
            # the same region of `out`. n indexes the batch half (cols =
            # b*512 + c*16 + w) and groups 0,1 / 2,3 cover b=0 / b=1, so
            # each half's accumulates overlap the other half's spatial work.
            spatial_group(0)
            spatial_group(1)
            for m2 in range(MT2):
                gemm2_pair(m2, 0)
            spatial_group(2)
            spatial_group(3)
            for m2 in range(MT2):
                gemm2_pair(m2, 1)


    nc.compile()
    _cache[key] = nc
    return nc


def kernel(x, w_spatial, b_spatial, w_spectral, b_spectral):
    x = np.ascontiguousarray(x, dtype=np.float32)
    w_spatial = np.asarray(w_spatial, dtype=np.float32)
    b_spatial = np.asarray(b_spatial, dtype=np.float32)
    w_spectral = np.asarray(w_spectral, dtype=np.float32)
    b_spectral = np.asarray(b_spectral, dtype=np.float32)

    F, G = _dft_constants()
    keep_f, keep_o = _spec_keep_idx()
    core_ids = list(range(N_CORES))
    tr = _trace_flag()

    # ---- launch 1: fold A^T = F^T @ W^T, sharded over H ----
    nc1 = _build_fold()
    w_t = np.ascontiguousarray(w_spectral[keep_o][:, keep_f].T).astype(np.float16)
    in1 = [{"f_sl": np.ascontiguousarray(F[:, HSL * i:HSL * (i + 1)]).astype(np.float16),
            "w_t": w_t} for i in core_ids]
    kw1 = {}
    if tr:
        d = os.environ.get("KERNEL_TRACE_DIR", "/tmp/ktrace") + "/fold"
        os.makedirs(d, exist_ok=True)
        kw1 = dict(trace=True, tmpdir=d)
    res1 = run_bass_kernel_spmd(nc1, in1, core_ids, **kw1)
    global LAST_FOLD_NS
    LAST_FOLD_NS = res1.exec_time_ns
    at_full = np.concatenate([res1.results[i]["at_sl"] for i in core_ids], axis=0)

    # ---- launch 2: main ----
    nc2 = _build_main()
    gt = np.ascontiguousarray(G.T).astype(np.float16)             # [384, 384]
    bspec = np.ascontiguousarray(b_spectral[keep_o].reshape(MO, 1))
    wsp = np.ascontiguousarray(
        w_spatial[:, :, :, 0].transpose(1, 2, 0).reshape(KSP, C)
    ).astype(np.float16)
    bsp = np.ascontiguousarray(b_spatial.reshape(C, 1))
    at16 = at_full.astype(np.float16)
    in2 = []
    for i in core_ids:
        xs = x[:, :, :, WS * i:WS * (i + 1)]                      # [B, C, H, WS]
        xti = np.ascontiguousarray(
            xs.transpose(2, 0, 1, 3).reshape(H, NCOL)).astype(np.float16)
        xspi = np.ascontiguousarray(
            xs.reshape(B, C, OUT_H, 7, WS).transpose(1, 3, 0, 2, 4)
            .reshape(KSP, NSP)).astype(np.float16)
        in2.append({"at": at16, "xt": xti, "gt": gt, "bspec": bspec,
                    "wsp": wsp, "bsp": bsp, "xsp": xspi})
    kw2 = {}
    if tr:
        d = os.environ.get("KERNEL_TRACE_DIR", "/tmp/ktrace") + "/main"
        os.makedirs(d, exist_ok=True)
        kw2 = dict(trace=True, tmpdir=d)
    res2 = run_bass_kernel_spmd(nc2, in2, core_ids, **kw2)
    global LAST_EXEC_NS
    LAST_EXEC_NS = res2.exec_time_ns

    # ---- unshard: per-core out [384, (b, c, ws)] -> [B, C, 384, W] ----
    outs = np.stack([res2.results[i]["out"].reshape(OUT_H, B, C, WS)
                     for i in core_ids], axis=3)                  # [384,B,C,8,WS]
    return np.ascontiguousarray(
        outs.reshape(OUT_H, B, C, W).transpose(1, 2, 0, 3)).astype(np.float32)



# revision 25
# speedup vs baseline: 2.0471x; 2.0471x over previous
"""Trainium2 Bass kernel for the FFTBlock problem (B=2, C=32, H=2688, W=128).

Math (reference):
  spatial  = relu(conv7x1_s7(x) + b_spatial)                        [B,C,384,W]
  spectral = irfft(relu(w_spectral @ rfft_concat(x) + b_spectral))  per (b,c,w)
  out = spatial + spectral

Weight preprocessing (host, exact): rfft/irfft along H are linear, so
  A = w_spectral_kept @ F  with F the (ortho) rfft matrix (dead rows/cols
  dropped) is a [384, 2688] matrix, computed on host via one FFT over the
  weight rows: A[o,:] = Re(fft(wre[o] - i*wim[o], n=H)) / sqrt(H) (exact
  to fp64 rounding). G: [384, 384] irfft matrix (constant). Then per
  column x_col (one (b,c,w) fiber):
    spectral_col = G @ relu(A @ x_col + b)

Device plan (8 NeuronCores, W sharded 8 x 16), single launch, column
layout (b, w, c) so both branches produce the same [t, (b,w,c)] tiles:
  GEMM1  conv[384, 1024] = A @ x      (at stationary, xt moving, fp16)
  SPAT   per (b,w) pair and t-tile: ps[t128, c32] += xsp_chunk^T @ wsp
         (xsp stationary fp8, wsp moving fp8 [225, 32]; row 224 of xsp is
         ones and row 224 of wsp is b_spatial -> bias via contraction).
         Spatial chunks are interleaved INTO the GEMM1 k-loop (own PSUM
         banks) and relu'd into so_big by ACT as their fp8 data arrives.
  ACT/DVE relu1 = relu(conv + bspec)  (split across engines)
  GEMM2  spec[384, 1024] = G @ relu1  (gt stationary, relu1 moving), into
         the PSUM banks GEMM1 freed; DVE/Pool add so += spec; store.
  out fp16, one DMA per column-quarter covering all three 128-row blocks.
"""

import os

import numpy as np
import ml_dtypes

import concourse.bacc as bacc
import concourse.mybir as mybir
import concourse.tile as tile
from concourse.bass_utils import run_bass_kernel_spmd
from concourse.alu_op_type import AluOpType

N_CORES = 8
B, C, H, W = 2, 32, 2688, 128
FREQ_IN = H // 2 + 1            # 1345
OUT_H = 384
FREQ_OUT = OUT_H // 2 + 1       # 193
MO = 2 * FREQ_OUT - 2           # 384 usable conv channels (2 dead dropped)
WS = W // N_CORES               # 16 width columns per core
NCOL = B * C * WS               # 1024 columns per core, order (b, w, c)
KSP = C * 7                     # 224 spatial reduction (+1 bias row)
T_H = OUT_H                     # 384 output rows (t)

F32 = mybir.dt.float32
F16 = mybir.dt.float16
F8 = mybir.dt.float8e4
FP8NP = ml_dtypes.float8_e4m3fn
RELU = mybir.ActivationFunctionType.Relu

_cache = {}
LAST_EXEC_NS = None


def _trace_flag():
    return bool(int(os.environ.get("KERNEL_TRACE", "0")))


def _g_matrix():
    """G [384, 384]: irfft (ortho) matrix with dead cols dropped."""
    if "G" in _cache:
        return _cache["G"]
    G_re = np.fft.irfft(np.eye(FREQ_OUT), n=OUT_H, axis=0, norm="ortho")
    G_im = np.fft.irfft(1j * np.eye(FREQ_OUT), n=OUT_H, axis=0, norm="ortho")
    G = np.concatenate([G_re, G_im[:, 1:FREQ_OUT - 1]], axis=1)
    _cache["G"] = np.ascontiguousarray(G, dtype=np.float32)   # [384, 384]
    return _cache["G"]


def _spec_keep_idx():
    """Kept rfft rows (of the 2690 concat) / output channels (of the 386)."""
    keep_f = list(range(FREQ_IN)) + [FREQ_IN + k for k in range(1, FREQ_IN - 1)]
    keep_o = list(range(FREQ_OUT)) + [FREQ_OUT + k for k in range(1, FREQ_OUT - 1)]
    return np.array(keep_f), np.array(keep_o)


def _fold_a(w_spectral):
    """A = W_kept @ F computed exactly via one FFT over weight rows.

    A[o, h] = sum_f wre[o,f] cos(2pi f h/H)/sqrt(H)
            - sum_f wim[o,f] sin(2pi f h/H)/sqrt(H)
            = Re{ fft(wre[o] - i*wim[o], n=H) } / sqrt(H)
    """
    keep_f, keep_o = _spec_keep_idx()
    Wk = np.asarray(w_spectral, dtype=np.float64)[keep_o][:, keep_f]
    wre = Wk[:, :FREQ_IN]
    wim = np.zeros((MO, FREQ_IN))
    wim[:, 1:FREQ_IN - 1] = Wk[:, FREQ_IN:]
    c = np.concatenate([wre - 1j * wim,
                        np.zeros((MO, H - FREQ_IN))], axis=1)
    return (np.real(np.fft.fft(c, axis=1)) / np.sqrt(H)).astype(np.float32)


# spatial chunk layout: (b,w)-pair counts per chunk; last chunk tiny so the
# end-of-stream -> spatial -> add -> out chain is short
SP_CH = [8, 8, 8, 6, 2]
SP_P0 = [0, 8, 16, 24, 30]


def _build_main():
    """Per core main kernel; out[384, 1024] fp16 in (t, (b, w, c)) layout."""
    if "main" in _cache:
        return _cache["main"]
    nc = bacc.Bacc("TRN2", target_bir_lowering=False, debug=False,
                   num_devices=N_CORES)
    at = nc.dram_tensor("at", [H, MO], F16, kind="ExternalInput").ap()
    xt = nc.dram_tensor("xt", [H, NCOL], F16, kind="ExternalInput").ap()
    gt = nc.dram_tensor("gt", [MO, T_H], F16, kind="ExternalInput").ap()
    bspec = nc.dram_tensor("bspec", [MO, 1], F32, kind="ExternalInput").ap()
    wsp = nc.dram_tensor("wsp", [KSP + 1, C], F8, kind="ExternalInput").ap()
    xsp = nc.dram_tensor("xsp", [KSP + 1, B * WS * T_H], F8,
                         kind="ExternalInput").ap()
    out = nc.dram_tensor("out", [T_H, NCOL], F16, kind="ExternalOutput").ap()

    KT1 = H // 128               # 21 k-tiles for GEMM1
    MT1 = MO // 128              # 3 m-tiles
    NT = 2                       # column halves (512 each)
    NQ = 4                       # column quarters (256 each)
    MT2 = T_H // 128             # 3 t-tiles

    with tile.TileContext(nc) as tc:
        with tc.tile_pool(name="const", bufs=1) as cst, \
             tc.tile_pool(name="xtp", bufs=1) as xtp, \
             tc.tile_pool(name="xspp", bufs=1) as xspp, \
             tc.tile_pool(name="relu", bufs=1) as rlp, \
             tc.tile_pool(name="outp", bufs=1) as outp, \
             tc.tile_pool(name="ps", bufs=1, space="PSUM") as pp:

            # ---- SBUF tiles ----
            AT_B = [1, 2, 4, 7, 7]       # at k-tile batches (ramp up)
            at_t, at_g, at_off = [], [], []
            o = 0
            for g, nb in enumerate(AT_B):
                ag = cst.tile([128, nb * MO], F16, tag=f"atg{g}", name=f"atg{g}")
                at_g.append(ag)
                at_off.append(o)
                for j in range(nb):
                    at_t.append(ag[:, MO * j:MO * (j + 1)])
                o += nb
            XT_B = [1, 2] + [3] * 6      # xt k-tile batches
            xt_t, xt_g, xt_off = [], [], []
            o = 0
            for g, nb in enumerate(XT_B):
                xg = xtp.tile([128, nb * NCOL], F16, tag=f"xtg{g}", name=f"xtg{g}")
                xt_g.append(xg)
                xt_off.append(o)
                for j in range(nb):
                    xt_t.append(xg[:, NCOL * j:NCOL * (j + 1)])
                o += nb

            def at_dma(g):
                k0, nb = at_off[g], AT_B[g]
                nc.sync.dma_start(
                    at_g[g][:], at[128 * k0:128 * (k0 + nb), :]
                    .rearrange("(k p) m -> p k m", p=128))

            def xt_dma(g):
                k0, nb = xt_off[g], XT_B[g]
                nc.sync.dma_start(
                    xt_g[g][:], xt[128 * k0:128 * (k0 + nb), :]
                    .rearrange("(k p) m -> p k m", p=128))

            gt_big = cst.tile([128, MT2 * T_H], F16, tag="gt_big", name="gt_big")
            gt_t = [gt_big[:, T_H * k:T_H * (k + 1)] for k in range(MT1)]
            bspec_big = cst.tile([128, MT1], F32, tag="bspec_big",
                                 name="bspec_big")
            bspec_t = [bspec_big[:, m:m + 1] for m in range(MT1)]
            wsp1 = cst.tile([128, C], F8, tag="wsp1", name="wsp1")
            wsp2 = cst.tile([KSP + 1 - 128, C], F8, tag="wsp2", name="wsp2")

            xsp1, xsp2 = [], []
            for c, npair in enumerate(SP_CH):
                cw = npair * T_H
                xsp1.append(xspp.tile([128, cw], F8, tag=f"xsp1_{c}",
                                      name=f"xsp1_{c}"))
                xsp2.append(xspp.tile([KSP + 1 - 128, cw], F8, tag=f"xsp2_{c}",
                                      name=f"xsp2_{c}"))

            def xsp_dma(c):
                c0 = SP_P0[c] * T_H
                cw = SP_CH[c] * T_H
                nc.sync.dma_start(xsp1[c][:], xsp[0:128, c0:c0 + cw])
                nc.sync.dma_start(xsp2[c][:], xsp[128:KSP + 1, c0:c0 + cw])

            # ---- DMA issue order: at/xt ramp, consts, xsp chunks woven
            # between the last xt groups ----
            xt_dma(0)
            at_dma(0)
            at_dma(1)
            xt_dma(1)
            at_dma(2)
            xt_dma(2)
            at_dma(3)
            xt_dma(3)
            at_dma(4)
            xt_dma(4)
            nc.sync.dma_start(gt_big[:],
                              gt[:].rearrange("(k p) m -> p k m", p=128))
            nc.sync.dma_start(bspec_big[:],
                              bspec[:].rearrange("(m p) one -> p m one", p=128))
            nc.sync.dma_start(wsp1[:], wsp[0:128, :])
            nc.sync.dma_start(wsp2[:], wsp[128:KSP + 1, :])
            xt_dma(5)
            xsp_dma(0)
            xt_dma(6)
            xsp_dma(1)
            xt_dma(7)
            xsp_dma(2)
            xsp_dma(3)
            xsp_dma(4)

            # ---- PSUM ----
            T = [[pp.tile([128, 512], F32, tag=f"g1m{m}n{n}", name=f"g1m{m}n{n}")
                  for n in range(NT)] for m in range(MT1)]
            S = [pp.tile([128, 512], F32, tag="spA", name="spA"),
                 pp.tile([128, 512], F32, tag="spB", name="spB")]

            relu_t = [rlp.tile([128, NCOL], F16, tag=f"relu{m}", name=f"relu{m}")
                      for m in range(MT1)]
            # single output tile, m2-major columns: so_big[:, m2*1024 + col]
            # -> one out DMA covers all three 128-row blocks of a col chunk
            so_big = outp.tile([128, MT2 * NCOL], F16, tag="so", name="so")
            so_t = [so_big[:, NCOL * m2:NCOL * (m2 + 1)] for m2 in range(MT2)]
            out_p = out.rearrange("(m p) c -> p m c", p=128)
            so_p = so_big[:].rearrange("p (m c) -> p m c", m=MT2)

            slot_i = [0]

            def sp_chunk(c):
                """Spatial matmuls + ACT relu for xsp chunk c (own PSUM)."""
                npair, p0 = SP_CH[c], SP_P0[c]
                csl = slice(32 * p0, 32 * (p0 + npair))
                for m2 in range(MT2):
                    slot = slot_i[0] % 4
                    slot_i[0] += 1
                    ps = S[slot // 2][:, 256 * (slot % 2):
                                      256 * (slot % 2) + 32 * npair]
                    t0 = 128 * m2
                    for j in range(npair):
                        cb = j * T_H + t0
                        nc.tensor.matmul(ps[:, 32 * j:32 * (j + 1)],
                                         xsp1[c][:, cb:cb + 128],
                                         wsp1[:], start=True, stop=False)
                        nc.tensor.matmul(ps[:, 32 * j:32 * (j + 1)],
                                         xsp2[c][:, cb:cb + 128],
                                         wsp2[:], start=False, stop=True)
                    nc.scalar.activation(so_t[m2][:, csl], ps[:], RELU)

            # ---- PE warmup: junk matmuls (on a memset tile) keep the PE
            # busy from ~1us so the p-state ramp completes before GEMM1's
            # real work arrives ----
            junk = rlp.tile([128, 512], F16, tag="junk", name="junk")
            nc.gpsimd.memset(junk[:], 0)
            for _ in range(6):
                nc.tensor.matmul(S[0][:], junk[:, 0:128], junk[:],
                                 start=True, stop=True)

            # ---- GEMM1 k-outer with spatial chunks woven in; last 3 k
            # iterations finish n=1 first so relu(n=1) can run early ----
            SP_AT = {14: 0, 16: 1}
            for k in range(KT1 - 3):
                for m in range(MT1):
                    msl = slice(128 * m, 128 * (m + 1))
                    for n in range(NT):
                        nc.tensor.matmul(T[m][n][:], at_t[k][:, msl],
                                         xt_t[k][:, 512 * n:512 * (n + 1)],
                                         start=(k == 0), stop=False)
                if k in SP_AT:
                    sp_chunk(SP_AT[k])
            for k in range(KT1 - 3, KT1):
                for m in range(MT1):
                    nc.tensor.matmul(T[m][1][:], at_t[k][:, 128 * m:128 * (m + 1)],
                                     xt_t[k][:, 512:1024],
                                     start=False, stop=(k == KT1 - 1))
            for m in range(MT1):
                nc.vector.tensor_scalar(relu_t[m][:, 512:1024], T[m][1][:],
                                        bspec_t[m][:], 0.0,
                                        AluOpType.add, AluOpType.max)
            sp_chunk(2)
            sp_chunk(3)
            for k in range(KT1 - 3, KT1):
                for m in range(MT1):
                    nc.tensor.matmul(T[m][0][:], at_t[k][:, 128 * m:128 * (m + 1)],
                                     xt_t[k][:, 0:512],
                                     start=False, stop=(k == KT1 - 1))
            # relu1 n=0 split: ACT m0/m1, DVE m2
            nc.scalar.activation(relu_t[0][:, 0:512], T[0][0][:],
                                 RELU, bias=bspec_t[0][:])
            nc.vector.tensor_scalar(relu_t[2][:, 0:512], T[2][0][:],
                                    bspec_t[2][:], 0.0,
                                    AluOpType.add, AluOpType.max)
            nc.scalar.activation(relu_t[1][:, 0:512], T[1][0][:],
                                 RELU, bias=bspec_t[1][:])
            sp_chunk(4)

            # ---- GEMM2 chunks (n=1 cols first, tiny final chunk), add,
            # store ----
            G2_CH = [(512, 768), (768, 1024), (0, 256), (256, 448), (448, 512)]
            out_q = [nc.sync, nc.scalar, nc.sync, nc.sync, nc.scalar]
            for qi, (lo, hi) in enumerate(G2_CH):
                qsl = slice(lo, hi)
                bk = lo // 512
                psl = slice(lo - 512 * bk, hi - 512 * bk)
                for m2 in range(MT2):
                    ps2 = T[m2][bk]
                    m2sl = slice(128 * m2, 128 * (m2 + 1))
                    for kk in range(MT1):
                        nc.tensor.matmul(ps2[:, psl], gt_t[kk][:, m2sl],
                                         relu_t[kk][:, qsl],
                                         start=(kk == 0), stop=(kk == MT1 - 1))
                    # Pool/GPSIMD cannot read PSUM on hw -> adds stay on DVE
                    nc.vector.tensor_add(so_t[m2][:, qsl], ps2[:, psl],
                                         so_t[m2][:, qsl])
                out_q[qi].dma_start(out_p[:, :, qsl], so_p[:, :, qsl])

    nc.compile()
    _cache["main"] = nc
    return nc


def kernel(x, w_spatial, b_spatial, w_spectral, b_spectral):
    x = np.ascontiguousarray(x, dtype=np.float32)
    w_spatial = np.asarray(w_spatial, dtype=np.float32)
    b_spatial = np.asarray(b_spatial, dtype=np.float32)
    w_spectral = np.asarray(w_spectral, dtype=np.float32)
    b_spectral = np.asarray(b_spectral, dtype=np.float32)

    keep_f, keep_o = _spec_keep_idx()
    core_ids = list(range(N_CORES))

    # host weight prep (exact, cheap): A via FFT, G constant
    A = _fold_a(w_spectral)                                   # [384, 2688]
    at16 = np.ascontiguousarray(A.T).astype(np.float16)       # [2688, 384]
    G = _g_matrix()
    gt = np.ascontiguousarray(G.T).astype(np.float16)         # [384, 384]
    bspec = np.ascontiguousarray(b_spectral[keep_o].reshape(MO, 1))
    wsp = np.concatenate([
        w_spatial[:, :, :, 0].transpose(1, 2, 0).reshape(KSP, C),
        b_spatial.reshape(1, C)], axis=0).astype(FP8NP)       # [225, 32]

    nc = _build_main()
    in2 = []
    for i in core_ids:
        xs = x[:, :, :, WS * i:WS * (i + 1)]                  # [B, C, H, WS]
        xti = np.ascontiguousarray(
            xs.transpose(2, 0, 3, 1).reshape(H, NCOL)).astype(np.float16)
        xspi = np.concatenate([
            xs.reshape(B, C, T_H, 7, WS).transpose(1, 3, 0, 4, 2)
            .reshape(KSP, B * WS * T_H),
            np.ones((1, B * WS * T_H), np.float32)], axis=0).astype(FP8NP)
        in2.append({"at": at16, "xt": xti, "gt": gt, "bspec": bspec,
                    "wsp": wsp, "xsp": xspi})
    kw = {}
    if _trace_flag():
        d = os.environ.get("KERNEL_TRACE_DIR", "/tmp/ktrace") + "/main"
        os.makedirs(d, exist_ok=True)
        kw = dict(trace=True, tmpdir=d)
    res = run_bass_kernel_spmd(nc, in2, core_ids, **kw)
    global LAST_EXEC_NS
    LAST_EXEC_NS = res.exec_time_ns

    # ---- unshard: per-core out [384, (b, w, c)] -> [B, C, 384, W] ----
    outs = np.stack([res.results[i]["out"].reshape(T_H, B, WS, C)
                     for i in core_ids], axis=2)              # [384,B,8,WS,C]
    return np.ascontiguousarray(
        outs.reshape(T_H, B, W, C).transpose(1, 3, 0, 2)).astype(np.float32)
